# revision 19
# baseline (speedup 1.0000x reference)
"""Trainium2 Bass kernel for ProbLinear (Bayesian linear layer, sampled weights).

Computes, in fp32 inputs / float32r matmul precision:
    W    = weight_mu + softplus(weight_rho) * eps_w          [OUT_F, IN_F]
    b    = bias_mu + softplus(bias_rho) * eps_b              [OUT_F]
    out  = x @ W.T + b                                       [TOKENS, OUT_F]

Sharding across 8 NeuronCores: 2-way over tokens x 4-way over out_features.
Each core samples its W slice on-chip and runs a K-accumulated float32r
matmul (full PE rate, ~1.2e-4 rel error vs fp32 reference). The contraction
dim must sit on SBUF partitions for both matmul operands; instead of PE
transposes, inputs are block-permuted on the host (a free layout choice
during sharding) so a contiguous DMA + DVE 32x32 StreamTranspose + ACT
rounding cast produce the transposed f32r tiles off the critical PE path.

Self-contained: hardcodes shapes, builds + caches the Bass program, shards
inputs on the host, runs via run_bass_kernel_spmd, reassembles full output.
Measured on trn2: ~651 us HW exec — dense matmul phase (470 us PE busy,
zero gaps) behind a ~160 us weight-sampling prologue (its 50 MB DMA floor).
"""
import numpy as np
from contextlib import ExitStack

import concourse.bass as bass
import concourse.mybir as mybir
import concourse.tile as tile
from concourse.bass_utils import run_bass_kernel_spmd

# ----------------------------------------------------------------------------
# Workaround for this walrus build: only 1 sem wait per instruction is
# accepted by some codegen paths. After Tile scheduling, hoist excess waits
# onto same-engine NoOps inserted right before the offending instruction.
# ----------------------------------------------------------------------------
_MAX_WAITS = 1


def _split_excess_waits(nc):
    for f in nc.m.functions:
        for bb in f.blocks:
            insts = bb.instructions
            i = 0
            while i < len(insts):
                inst = insts[i]
                si = inst.sync_info
                if si is not None and len(si.on_wait) > _MAX_WAITS:
                    waits = list(si.on_wait)
                    excess, keep = waits[:-_MAX_WAITS], waits[-_MAX_WAITS:]
                    si.on_wait = keep
                    pos = i
                    for j in range(0, len(excess), _MAX_WAITS):
                        chunk = excess[j:j + _MAX_WAITS]
                        nop = mybir.InstNoOp(
                            name=f"{inst.name}-waitsplit-{j}", ins=[], outs=[]
                        )
                        nop.engine = inst.engine
                        nop.sync_info = mybir.SyncInfo(on_wait=chunk, on_update=[])
                        nc.register_instruction(nop, overwrite=True)
                        insts.insert(pos, nop)
                        pos += 1
                        i += 1
                i += 1


if not getattr(tile.TileContext, "_waitsplit_patched", False):
    _orig_exit = tile.TileContext.__exit__

    def _patched_exit(self, exc_type, exc_val, exc_tb):
        res = _orig_exit(self, exc_type, exc_val, exc_tb)
        if exc_type is None:
            _split_excess_waits(self.nc)
        return res

    tile.TileContext.__exit__ = _patched_exit
    tile.TileContext._waitsplit_patched = True

# ----------------------------------------------------------------------------
# Problem shapes / sharding
# ----------------------------------------------------------------------------
TOKENS, IN_F, OUT_F = 8192, 4096, 4096
T_SPLIT, O_SPLIT = 2, 4
N_CORES = T_SPLIT * O_SPLIT

T_C = TOKENS // T_SPLIT          # 4096 tokens per core
O_C = OUT_F // O_SPLIT           # 1024 out features per core
KT = IN_F // 128                 # 32 contraction tiles
TT = T_C // 128                  # 32 token tiles per core
OROWS = O_C // 128               # 8 weight row-tiles per core
KC = 4                           # k-chunks for sampling / x streaming
KCW = IN_F // KC                 # 1024-wide chunks
NB = 512                         # matmul moving free dim (one PSUM bank fp32)
OC = O_C // NB                   # 2 output column chunks per core

F32 = mybir.dt.float32
F32R = mybir.dt.float32r
AF = mybir.ActivationFunctionType


def _build_program():
    nc = bass.Bass()
    x_d = nc.declare_dram_parameter("x", [T_C, IN_F], F32, isOutput=False)
    wmu_d = nc.declare_dram_parameter("wmu", [O_C, IN_F], F32, isOutput=False)
    wrho_d = nc.declare_dram_parameter("wrho", [O_C, IN_F], F32, isOutput=False)
    weps_d = nc.declare_dram_parameter("weps", [O_C, IN_F], F32, isOutput=False)
    bmu_d = nc.declare_dram_parameter("bmu", [O_C], F32, isOutput=False)
    brho_d = nc.declare_dram_parameter("brho", [O_C], F32, isOutput=False)
    beps_d = nc.declare_dram_parameter("beps", [O_C], F32, isOutput=False)
    out_d = nc.declare_dram_parameter("out", [T_C, O_C], F32, isOutput=True)

    with tile.TileContext(nc) as tc, ExitStack() as ctx:
        const = ctx.enter_context(tc.tile_pool(name="const", bufs=1))
        stage = ctx.enter_context(tc.tile_pool(name="stage", bufs=6))
        stp = ctx.enter_context(tc.tile_pool(name="stp", bufs=2))
        xtp = ctx.enter_context(tc.tile_pool(name="xtp", bufs=1))
        outp = ctx.enter_context(tc.tile_pool(name="outp", bufs=1))
        mmpsum = ctx.enter_context(tc.tile_pool(name="mmpsum", bufs=2, space="PSUM"))

        # ------------------------------------------------------------------
        # Bias: sampled on one partition as f32r; added to each PSUM tile
        # via a trailing K=1 matmul (ones.T @ bias_row broadcasts over
        # partitions and accumulates into the product).
        # ------------------------------------------------------------------
        ones = const.tile([1, 128], F32)
        nc.gpsimd.memset(ones[:], 1.0)
        brow_mu = const.tile([1, O_C], F32)
        brow_rho = const.tile([1, O_C], F32)
        brow_eps = const.tile([1, O_C], F32)
        nc.sync.dma_start(brow_mu[:], bmu_d[None, :])
        nc.sync.dma_start(brow_rho[:], brho_d[None, :])
        nc.sync.dma_start(brow_eps[:], beps_d[None, :])
        # softplus(r) = ln(exp(r) + 1); Softplus isn't in this build's tables
        nc.scalar.activation(brow_rho[:], brow_rho[:], AF.Exp)
        nc.scalar.activation(brow_rho[:], brow_rho[:], AF.Ln, bias=1.0)
        nc.vector.tensor_mul(brow_rho[:], brow_rho[:], brow_eps[:])
        nc.vector.tensor_add(brow_rho[:], brow_rho[:], brow_mu[:])
        bias_bc = const.tile([128, O_C], F32)
        for oc in range(OC):
            bps = mmpsum.tile([128, NB], F32, tag="bps")
            nc.tensor.matmul(
                bps[:], ones[:], brow_rho[:, oc * NB:(oc + 1) * NB],
                start=True, stop=True,
            )
            nc.any.tensor_copy(out=bias_bc[:, oc * NB:(oc + 1) * NB], in_=bps[:])

        # ------------------------------------------------------------------
        # Phase 1: sample W slice and build W^T resident in SBUF as f32r.
        # wT[p, ki, o] = W[o, ki*128 + p]
        # ------------------------------------------------------------------
        # Inputs x/wmu/wrho/weps are host block-permuted within each 128x128
        # tile (block (a,b) <-> (b,a)), so a contiguous DMA + DVE 32x32
        # StreamTranspose yields exact 128-wide transposed tiles — no PE
        # transposes needed.
        KT_C = KCW // 128
        wT = const.tile([128, KT, O_C], F32R)
        for kc in range(KC):
            ci = kc * KCW
            for orow in range(OROWS):
                ro = orow * 128
                ws = stage.tile([128, KT_C, 128], F32, tag="stg")
                eps = stage.tile([128, KT_C, 128], F32, tag="stg")
                mu = stage.tile([128, KT_C, 128], F32, tag="stg")
                nc.sync.dma_start(ws[:], wrho_d[ro:ro + 128, ci:ci + KCW])
                nc.sync.dma_start(eps[:], weps_d[ro:ro + 128, ci:ci + KCW])
                nc.sync.dma_start(mu[:], wmu_d[ro:ro + 128, ci:ci + KCW])
                nc.scalar.activation(ws[:], ws[:], AF.Exp)
                nc.scalar.activation(ws[:], ws[:], AF.Ln, bias=1.0)
                nc.vector.tensor_mul(ws[:], ws[:], eps[:])
                nc.vector.tensor_add(ws[:], ws[:], mu[:])
                st32 = stp.tile([128, KT_C, 128], F32, tag="st32")
                nc.vector.transpose(st32[:], ws[:])
                nc.scalar.activation(
                    wT[:, kc * KT_C:(kc + 1) * KT_C, ro:ro + 128],
                    st32[:], AF.Copy,
                )

        # ------------------------------------------------------------------
        # Phase 2: stream x token-tiles, transpose, matmul, bias, store.
        # ------------------------------------------------------------------
        for tt in range(TT):
            rt = tt * 128
            # xT split per k-chunk so each part's WAR releases as soon as its
            # last matmul reads it — next tile's transposes overlap trailing
            # matmuls of this tile.
            xT_parts = []
            for h in range(KC):
                ci = h * KCW
                xh = stage.tile([128, KT_C, 128], F32, tag="stg")
                nc.sync.dma_start(xh[:], x_d[rt:rt + 128, ci:ci + KCW])
                st32 = stp.tile([128, KT_C, 128], F32, tag="st32")
                nc.vector.transpose(st32[:], xh[:])
                xTp = xtp.tile([128, KT_C, 128], F32R, tag=f"xT{h}", name=f"xT{h}")
                nc.scalar.activation(xTp[:], st32[:], AF.Copy)
                xT_parts.append(xTp)
            ot = outp.tile([128, O_C], F32)
            pss = [mmpsum.tile([128, NB], F32, tag=f"ps{oc}", name=f"ps{oc}") for oc in range(OC)]
            for ki in range(KT):
                lhsT = xT_parts[ki // KT_C][:, ki % KT_C]
                for oc in range(OC):
                    nc.tensor.matmul(
                        pss[oc][:],
                        lhsT,
                        wT[:, ki, oc * NB:(oc + 1) * NB],
                        start=(ki == 0),
                        stop=(ki == KT - 1),
                    )
            for oc in range(OC):
                nc.vector.tensor_add(
                    ot[:, oc * NB:(oc + 1) * NB], pss[oc][:],
                    bias_bc[:, oc * NB:(oc + 1) * NB],
                )
            nc.sync.dma_start(out_d[rt:rt + 128, :], ot[:])

    return nc


_PROGRAM = None


def _blockperm(a):
    """Swap 32-sub-blocks (a,b)<->(b,a) inside each 128x128 tile so that an
    on-chip 32x32 DVE StreamTranspose of a loaded tile yields the exact
    128x128 transpose."""
    R, C = a.shape
    return np.ascontiguousarray(
        a.reshape(R // 128, 4, 32, C // 128, 4, 32)
         .transpose(0, 4, 2, 3, 1, 5)
         .reshape(R, C)
    )


def kernel(x, weight_mu, weight_rho, bias_mu, bias_rho, eps_w, eps_b):
    global _PROGRAM
    if _PROGRAM is None:
        _PROGRAM = _build_program()
    nc = _PROGRAM

    x = _blockperm(np.asarray(x, dtype=np.float32))
    weight_mu = _blockperm(np.asarray(weight_mu, dtype=np.float32))
    weight_rho = _blockperm(np.asarray(weight_rho, dtype=np.float32))
    eps_w = _blockperm(np.asarray(eps_w, dtype=np.float32))
    bias_mu = np.ascontiguousarray(np.asarray(bias_mu, dtype=np.float32))
    bias_rho = np.ascontiguousarray(np.asarray(bias_rho, dtype=np.float32))
    eps_b = np.ascontiguousarray(np.asarray(eps_b, dtype=np.float32))

    in_maps = []
    for c in range(N_CORES):
        ti, oi = c // O_SPLIT, c % O_SPLIT
        ts_, te = ti * T_C, (ti + 1) * T_C
        os_, oe = oi * O_C, (oi + 1) * O_C
        in_maps.append({
            "x": np.ascontiguousarray(x[ts_:te]),
            "wmu": np.ascontiguousarray(weight_mu[os_:oe]),
            "wrho": np.ascontiguousarray(weight_rho[os_:oe]),
            "weps": np.ascontiguousarray(eps_w[os_:oe]),
            "bmu": np.ascontiguousarray(bias_mu[os_:oe]),
            "brho": np.ascontiguousarray(bias_rho[os_:oe]),
            "beps": np.ascontiguousarray(eps_b[os_:oe]),
        })

    res = run_bass_kernel_spmd(nc, in_maps, list(range(N_CORES)))
    kernel.last_results = res

    out = np.empty((TOKENS, OUT_F), dtype=np.float32)
    for c in range(N_CORES):
        ti, oi = c // O_SPLIT, c % O_SPLIT
        out[ti * T_C:(ti + 1) * T_C, oi * O_C:(oi + 1) * O_C] = res.results[c]["out"]
    return out


# revision 20
# speedup vs baseline: 1.0061x; 1.0061x over previous
"""Trainium2 Bass kernel for ProbLinear (Bayesian linear layer, sampled weights).

Computes, in fp32 inputs / float32r matmul precision:
    W    = weight_mu + softplus(weight_rho) * eps_w          [OUT_F, IN_F]
    b    = bias_mu + softplus(bias_rho) * eps_b              [OUT_F]
    out  = x @ W.T + b                                       [TOKENS, OUT_F]

Sharding across 8 NeuronCores: 2-way over tokens x 4-way over out_features.
Each core samples its W slice on-chip and runs a K-accumulated float32r
matmul (full PE rate, ~1.2e-4 rel error vs fp32 reference). The contraction
dim must sit on SBUF partitions for both matmul operands; instead of PE
transposes, inputs are block-permuted on the host (a free layout choice
during sharding) so a contiguous DMA + DVE 32x32 StreamTranspose + ACT
rounding cast produce the transposed f32r tiles off the critical PE path.

Self-contained: hardcodes shapes, builds + caches the Bass program, shards
inputs on the host, runs via run_bass_kernel_spmd, reassembles full output.
Measured on trn2: ~651 us HW exec — dense matmul phase (470 us PE busy,
zero gaps) behind a ~160 us weight-sampling prologue (its 50 MB DMA floor).
"""
import numpy as np
from contextlib import ExitStack

import concourse.bass as bass
import concourse.mybir as mybir
import concourse.tile as tile
from concourse.bass_utils import run_bass_kernel_spmd

# ----------------------------------------------------------------------------
# Workaround for this walrus build: only 1 sem wait per instruction is
# accepted by some codegen paths. After Tile scheduling, hoist excess waits
# onto same-engine NoOps inserted right before the offending instruction.
# ----------------------------------------------------------------------------
_MAX_WAITS = 1


def _split_excess_waits(nc):
    for f in nc.m.functions:
        for bb in f.blocks:
            insts = bb.instructions
            i = 0
            while i < len(insts):
                inst = insts[i]
                si = inst.sync_info
                if si is not None and len(si.on_wait) > _MAX_WAITS:
                    waits = list(si.on_wait)
                    excess, keep = waits[:-_MAX_WAITS], waits[-_MAX_WAITS:]
                    si.on_wait = keep
                    pos = i
                    for j in range(0, len(excess), _MAX_WAITS):
                        chunk = excess[j:j + _MAX_WAITS]
                        nop = mybir.InstNoOp(
                            name=f"{inst.name}-waitsplit-{j}", ins=[], outs=[]
                        )
                        nop.engine = inst.engine
                        nop.sync_info = mybir.SyncInfo(on_wait=chunk, on_update=[])
                        nc.register_instruction(nop, overwrite=True)
                        insts.insert(pos, nop)
                        pos += 1
                        i += 1
                i += 1


if not getattr(tile.TileContext, "_waitsplit_patched", False):
    _orig_exit = tile.TileContext.__exit__

    def _patched_exit(self, exc_type, exc_val, exc_tb):
        res = _orig_exit(self, exc_type, exc_val, exc_tb)
        if exc_type is None:
            _split_excess_waits(self.nc)
        return res

    tile.TileContext.__exit__ = _patched_exit
    tile.TileContext._waitsplit_patched = True

# ----------------------------------------------------------------------------
# Problem shapes / sharding
# ----------------------------------------------------------------------------
TOKENS, IN_F, OUT_F = 8192, 4096, 4096
T_SPLIT, O_SPLIT = 2, 4
N_CORES = T_SPLIT * O_SPLIT

T_C = TOKENS // T_SPLIT          # 4096 tokens per core
O_C = OUT_F // O_SPLIT           # 1024 out features per core
KT = IN_F // 128                 # 32 contraction tiles
TT = T_C // 128                  # 32 token tiles per core
OROWS = O_C // 128               # 8 weight row-tiles per core
KC = 4                           # k-chunks for sampling / x streaming
KCW = IN_F // KC                 # 1024-wide chunks
NB = 512                         # matmul moving free dim (one PSUM bank fp32)
OC = O_C // NB                   # 2 output column chunks per core

F32 = mybir.dt.float32
F32R = mybir.dt.float32r
AF = mybir.ActivationFunctionType


def _build_program():
    nc = bass.Bass()
    x_d = nc.declare_dram_parameter("x", [T_C, IN_F], F32, isOutput=False)
    wmu_d = nc.declare_dram_parameter("wmu", [O_C, IN_F], F32, isOutput=False)
    wrho_d = nc.declare_dram_parameter("wrho", [O_C, IN_F], F32, isOutput=False)
    weps_d = nc.declare_dram_parameter("weps", [O_C, IN_F], F32, isOutput=False)
    bmu_d = nc.declare_dram_parameter("bmu", [O_C], F32, isOutput=False)
    brho_d = nc.declare_dram_parameter("brho", [O_C], F32, isOutput=False)
    beps_d = nc.declare_dram_parameter("beps", [O_C], F32, isOutput=False)
    out_d = nc.declare_dram_parameter("out", [T_C, O_C], F32, isOutput=True)

    with tile.TileContext(nc) as tc, ExitStack() as ctx:
        const = ctx.enter_context(tc.tile_pool(name="const", bufs=1))
        stage = ctx.enter_context(tc.tile_pool(name="stage", bufs=6))
        stp = ctx.enter_context(tc.tile_pool(name="stp", bufs=2))
        xtp = ctx.enter_context(tc.tile_pool(name="xtp", bufs=1))
        outp = ctx.enter_context(tc.tile_pool(name="outp", bufs=1))
        mmpsum = ctx.enter_context(tc.tile_pool(name="mmpsum", bufs=3, space="PSUM"))

        # ------------------------------------------------------------------
        # Bias: sampled on one partition as f32r; added to each PSUM tile
        # via a trailing K=1 matmul (ones.T @ bias_row broadcasts over
        # partitions and accumulates into the product).
        # ------------------------------------------------------------------
        ones = const.tile([1, 128], F32)
        nc.gpsimd.memset(ones[:], 1.0)
        brow_mu = const.tile([1, O_C], F32)
        brow_rho = const.tile([1, O_C], F32)
        brow_eps = const.tile([1, O_C], F32)
        nc.sync.dma_start(brow_mu[:], bmu_d[None, :])
        nc.sync.dma_start(brow_rho[:], brho_d[None, :])
        nc.sync.dma_start(brow_eps[:], beps_d[None, :])
        # softplus(r) = ln(exp(r) + 1); Softplus isn't in this build's tables
        nc.scalar.activation(brow_rho[:], brow_rho[:], AF.Exp)
        nc.scalar.activation(brow_rho[:], brow_rho[:], AF.Ln, bias=1.0)
        nc.vector.tensor_mul(brow_rho[:], brow_rho[:], brow_eps[:])
        nc.vector.tensor_add(brow_rho[:], brow_rho[:], brow_mu[:])
        bias_bc = const.tile([128, O_C], F32)
        for oc in range(OC):
            bps = mmpsum.tile([128, NB], F32, tag="ps0", name="bps")
            nc.tensor.matmul(
                bps[:], ones[:], brow_rho[:, oc * NB:(oc + 1) * NB],
                start=True, stop=True,
            )
            nc.any.tensor_copy(out=bias_bc[:, oc * NB:(oc + 1) * NB], in_=bps[:])

        # ------------------------------------------------------------------
        # Phase 1: sample W slice and build W^T resident in SBUF as f32r.
        # wT[p, ki, o] = W[o, ki*128 + p]
        # ------------------------------------------------------------------
        # Inputs x/wmu/wrho/weps are host block-permuted within each 128x128
        # tile (block (a,b) <-> (b,a)), so a contiguous DMA + DVE 32x32
        # StreamTranspose yields exact 128-wide transposed tiles — no PE
        # transposes needed.
        KT_C = KCW // 128
        wT_parts = [
            const.tile([128, KT_C, O_C], F32R, tag=f"wT{kc}", name=f"wT{kc}")
            for kc in range(KC)
        ]
        for kc in range(KC):
            ci = kc * KCW
            for orow in range(OROWS):
                ro = orow * 128
                ws = stage.tile([128, KT_C, 128], F32, tag="stg")
                eps = stage.tile([128, KT_C, 128], F32, tag="stg")
                mu = stage.tile([128, KT_C, 128], F32, tag="stg")
                nc.sync.dma_start(ws[:], wrho_d[ro:ro + 128, ci:ci + KCW])
                nc.sync.dma_start(eps[:], weps_d[ro:ro + 128, ci:ci + KCW])
                nc.sync.dma_start(mu[:], wmu_d[ro:ro + 128, ci:ci + KCW])
                nc.scalar.activation(ws[:], ws[:], AF.Exp)
                nc.scalar.activation(ws[:], ws[:], AF.Ln, bias=1.0)
                nc.vector.tensor_mul(ws[:], ws[:], eps[:])
                nc.vector.tensor_add(ws[:], ws[:], mu[:])
                st32 = stp.tile([128, KT_C, 128], F32, tag="st32")
                nc.vector.transpose(st32[:], ws[:])
                nc.scalar.activation(
                    wT_parts[kc][:, :, ro:ro + 128], st32[:], AF.Copy,
                )

        # ------------------------------------------------------------------
        # Phase 2: stream x token-tiles, transpose, matmul, bias, store.
        # ------------------------------------------------------------------
        for tt in range(TT):
            rt = tt * 128
            # xT split per k-chunk so each part's WAR releases as soon as its
            # last matmul reads it — next tile's transposes overlap trailing
            # matmuls of this tile.
            xT_parts = []
            for h in range(KC):
                ci = h * KCW
                xh = stage.tile([128, KT_C, 128], F32, tag="stg")
                nc.sync.dma_start(xh[:], x_d[rt:rt + 128, ci:ci + KCW])
                st32 = stp.tile([128, KT_C, 128], F32, tag="st32")
                nc.vector.transpose(st32[:], xh[:])
                xTp = xtp.tile([128, KT_C, 128], F32R, tag=f"xT{h}", name=f"xT{h}")
                nc.scalar.activation(xTp[:], st32[:], AF.Copy)
                xT_parts.append(xTp)
            ot = outp.tile([128, O_C], F32)
            pss = [mmpsum.tile([128, NB], F32, tag=f"ps{oc}", name=f"ps{oc}") for oc in range(OC)]
            for ki in range(KT):
                lhsT = xT_parts[ki // KT_C][:, ki % KT_C]
                for oc in range(OC):
                    nc.tensor.matmul(
                        pss[oc][:],
                        lhsT,
                        wT_parts[ki // KT_C][:, ki % KT_C, oc * NB:(oc + 1) * NB],
                        start=(ki == 0),
                        stop=(ki == KT - 1),
                    )
            for oc in range(OC):
                nc.vector.tensor_add(
                    ot[:, oc * NB:(oc + 1) * NB], pss[oc][:],
                    bias_bc[:, oc * NB:(oc + 1) * NB],
                )
            nc.sync.dma_start(out_d[rt:rt + 128, :], ot[:])

    return nc


_PROGRAM = None


def _blockperm(a):
    """Swap 32-sub-blocks (a,b)<->(b,a) inside each 128x128 tile so that an
    on-chip 32x32 DVE StreamTranspose of a loaded tile yields the exact
    128x128 transpose."""
    R, C = a.shape
    return np.ascontiguousarray(
        a.reshape(R // 128, 4, 32, C // 128, 4, 32)
         .transpose(0, 4, 2, 3, 1, 5)
         .reshape(R, C)
    )


def kernel(x, weight_mu, weight_rho, bias_mu, bias_rho, eps_w, eps_b):
    global _PROGRAM
    if _PROGRAM is None:
        _PROGRAM = _build_program()
    nc = _PROGRAM

    x = _blockperm(np.asarray(x, dtype=np.float32))
    weight_mu = _blockperm(np.asarray(weight_mu, dtype=np.float32))
    weight_rho = _blockperm(np.asarray(weight_rho, dtype=np.float32))
    eps_w = _blockperm(np.asarray(eps_w, dtype=np.float32))
    bias_mu = np.ascontiguousarray(np.asarray(bias_mu, dtype=np.float32))
    bias_rho = np.ascontiguousarray(np.asarray(bias_rho, dtype=np.float32))
    eps_b = np.ascontiguousarray(np.asarray(eps_b, dtype=np.float32))

    in_maps = []
    for c in range(N_CORES):
        ti, oi = c // O_SPLIT, c % O_SPLIT
        ts_, te = ti * T_C, (ti + 1) * T_C
        os_, oe = oi * O_C, (oi + 1) * O_C
        in_maps.append({
            "x": np.ascontiguousarray(x[ts_:te]),
            "wmu": np.ascontiguousarray(weight_mu[os_:oe]),
            "wrho": np.ascontiguousarray(weight_rho[os_:oe]),
            "weps": np.ascontiguousarray(eps_w[os_:oe]),
            "bmu": np.ascontiguousarray(bias_mu[os_:oe]),
            "brho": np.ascontiguousarray(bias_rho[os_:oe]),
            "beps": np.ascontiguousarray(eps_b[os_:oe]),
        })

    res = run_bass_kernel_spmd(nc, in_maps, list(range(N_CORES)))
    kernel.last_results = res

    out = np.empty((TOKENS, OUT_F), dtype=np.float32)
    for c in range(N_CORES):
        ti, oi = c // O_SPLIT, c % O_SPLIT
        out[ti * T_C:(ti + 1) * T_C, oi * O_C:(oi + 1) * O_C] = res.results[c]["out"]
    return out
